# revision 1
# baseline (speedup 1.0000x reference)
"""Causal self-attention (B=4, T=1024, C=2048, H=16, rotary) on 8 trn2 cores.

Sharding: core c = 2*b + g handles batch b, head-group g (heads 8g..8g+7).
 - QKV projection computed in transposed layout: Q^T/K^T = [d_channels, T],
   V in natural [T, d_channels] layout (for the att@V contraction).
 - RoPE via host-precomputed full-height cos/sin tables; the rotate-half
   partition swap runs on the PE as a permutation matmul.
 - Scores computed transposed S^T = K_tile^T . Q -> [k, q]; softmax without
   max-subtraction (logits are ~N(0,1); exp can't overflow); causal masking
   via multiplicative 0/1 bf16 masks on diagonal-straddling blocks only;
   denominator via ones-vector matmul accumulated over k-tiles.
 - att@V accumulated in PSUM over k-tiles -> y^T [d, q]; normalized by
   broadcast reciprocal row-sums while copying PSUM->SBUF.
 - AllGather (pairs sharing a batch) of y^T halves, then each core computes
   the full c_proj for half of the output columns, + bias.
All matmuls in bf16 (fp32 PSUM accumulation).
"""

import math

import numpy as np
import ml_dtypes

BF16 = ml_dtypes.bfloat16

B, T, C = 4, 1024, 2048
H = 16  # total heads
D = C // H  # 128 head dim
HG = 8  # heads per group (per core)
N_CORES = 8
ROPE_BASE = 10000.0

TUNE = {
    "chunk_order": (0, 2, 4, 1, 3, 5),
    "ps_bufs": 3,
    "ps_s_bufs": 2,
    "ps_y_bufs": 2,
    "p_sb_bufs": 6,
}

_PROGRAM_CACHE = {}


def _build_program(num_devices=N_CORES, collective=True, reps=1):
    import concourse.mybir as mybir
    import concourse.tile as tile
    from concourse import bacc
    from concourse.bass import ts

    f32 = mybir.dt.float32
    bf16 = mybir.dt.bfloat16
    AF = mybir.ActivationFunctionType

    nc = bacc.Bacc(trn_type="TRN2", num_devices=num_devices, debug=False)

    # ---- per-core I/O ----
    xT = nc.dram_tensor("xT", [C, T], bf16, kind="ExternalInput")  # x[b].T
    wqkv = nc.dram_tensor("wqkv", [C, 3 * HG * D], bf16, kind="ExternalInput")
    bqk = nc.dram_tensor("bqk", [128, 16], f32, kind="ExternalInput")
    bv = nc.dram_tensor("bv", [1, HG * D], f32, kind="ExternalInput")
    # full-height rope tables: cos2 = [cos; cos], sin2 = [-sin; sin]
    cosT = nc.dram_tensor("cosT", [D, T], bf16, kind="ExternalInput")
    sinT = nc.dram_tensor("sinT", [D, T], bf16, kind="ExternalInput")
    maskT = nc.dram_tensor("maskT", [128, 4, 512], bf16, kind="ExternalInput")
    # half-swap permutation: perm[j2, j] = 1 iff j2 == (j + 64) % 128
    perm = nc.dram_tensor("perm", [128, 128], bf16, kind="ExternalInput")
    wproj = nc.dram_tensor("wproj", [C, C // 2], bf16, kind="ExternalInput")
    bproj = nc.dram_tensor("bproj", [1, C // 2], f32, kind="ExternalInput")
    out = nc.dram_tensor("out", [T, C // 2], f32, kind="ExternalOutput")

    xT_r = xT.ap().rearrange("(ct p) t -> p ct t", p=128)  # [128, 16, 1024]
    wqkv_r = wqkv.ap().rearrange("(ct p) j -> p ct j", p=128)  # [128, 16, 3072]
    wproj_r = wproj.ap().rearrange("(jt p) c -> p jt c", p=128)  # [128, 16, 1024]

    scale = 1.0 / math.sqrt(D)

    with tile.TileContext(nc) as tc:
        with (
            tc.tile_pool(name="const", bufs=1) as const,
            tc.tile_pool(name="persist", bufs=1) as persist,
            tc.tile_pool(name="ps", bufs=TUNE["ps_bufs"], space="PSUM") as pspool,
            tc.tile_pool(
                name="ps_s", bufs=TUNE["ps_s_bufs"], space="PSUM"
            ) as ps_s_pool,
            tc.tile_pool(
                name="ps_y", bufs=TUNE["ps_y_bufs"], space="PSUM"
            ) as ps_y_pool,
            tc.tile_pool(name="ps_sum", bufs=1, space="PSUM") as ps_sum_pool,
            tc.tile_pool(name="work", bufs=4) as work,
            tc.tile_pool(name="dram", bufs=1, space="DRAM") as drampool,
        ):
            # ---- constants ----
            cos_sb = const.tile([128, T], bf16)
            nc.sync.dma_start(out=cos_sb, in_=cosT.ap())
            sin_sb = const.tile([128, T], bf16)
            nc.sync.dma_start(out=sin_sb, in_=sinT.ap())
            mask_sb = const.tile([128, 4, 512], bf16)
            nc.sync.dma_start(out=mask_sb, in_=maskT.ap())
            perm_sb = const.tile([128, 128], bf16)
            nc.sync.dma_start(out=perm_sb, in_=perm.ap())
            ones_sb = const.tile([128, 1], bf16)
            nc.vector.memset(ones_sb, 1.0)
            bqk_sb = const.tile([128, 16], f32)
            nc.sync.dma_start(out=bqk_sb, in_=bqk.ap())
            bv_bc = const.tile([128, HG * D], f32)
            nc.sync.dma_start(out=bv_bc, in_=bv.ap().to_broadcast([128, HG * D]))
            bp_bc = const.tile([128, C // 2], f32)
            nc.sync.dma_start(out=bp_bc, in_=bproj.ap().to_broadcast([128, C // 2]))

            # ---- persistent activations (reused across reps) ----
            qf = persist.tile([128, HG, T], bf16)  # [d, h, t] rotated Q^T
            kf = persist.tile([128, HG, T], bf16)  # [d, h, t] rotated K^T
            v_all = persist.tile([128, 8, HG * D], bf16)  # [t_in, tt, j]
            yT = persist.tile([128, HG, T], bf16)  # [d, h, t] normalized att out

            for rep in range(reps):
                _emit_once(
                    nc, tc, mybir, ts, f32, bf16, AF, scale, collective, rep,
                    xT_r, wqkv_r, wproj_r, out,
                    cos_sb, sin_sb, mask_sb, perm_sb, ones_sb,
                    bqk_sb, bv_bc, bp_bc,
                    qf, kf, v_all, yT,
                    pspool, ps_s_pool, ps_y_pool, ps_sum_pool, work, drampool,
                )

    nc.finalize()
    return nc


def _emit_once(
    nc, tc, mybir, ts, f32, bf16, AF, scale, collective, rep,
    xT_r, wqkv_r, wproj_r, out,
    cos_sb, sin_sb, mask_sb, perm_sb, ones_sb, bqk_sb, bv_bc, bp_bc,
    qf, kf, v_all, yT,
    pspool, ps_s_pool, ps_y_pool, ps_sum_pool, work, drampool,
):
    # =========== Phase A: QKV projection (+bias, +RoPE) ===========
    with (
        tc.tile_pool(name=f"xpool{rep}", bufs=1) as xpool,
        tc.tile_pool(name=f"wpool{rep}", bufs=2) as wpool,
    ):
        xs = xpool.tile([128, 16, T], bf16, name="xs")
        for ct in range(16):  # per-ct so first matmuls start early
            nc.sync.dma_start(out=xs[:, ct, :], in_=xT_r[:, ct, :])

        # order q0,k0,v0 first so heads 0-3 complete early and their
        # attention overlaps the rest of the QKV projection
        for chunk in TUNE["chunk_order"]:
            wt = wpool.tile([128, 16, 512], bf16, tag="wt", name="wt")
            for ct in range(16):
                nc.sync.dma_start(
                    out=wt[:, ct, :],
                    in_=wqkv_r[:, ct, chunk * 512 : (chunk + 1) * 512],
                )
            if chunk < 4:  # Q or K, output transposed [j, t]
                for jj in range(4):
                    jt = chunk * 4 + jj  # 0..15 (q: 0-7, k: 8-15)
                    h = jt % 8
                    dest_all = qf if jt < 8 else kf
                    for th in range(2):  # t halves of 512
                        ps = pspool.tile([128, 512], f32, tag="ps", name="ps")
                        for ct in range(16):
                            nc.tensor.matmul(
                                ps,
                                lhsT=wt[:, ct, jj * 128 : (jj + 1) * 128],
                                rhs=xs[:, ct, ts(th, 512)],
                                start=(ct == 0),
                                stop=(ct == 15),
                            )
                        raw = work.tile([128, 512], bf16, tag="raw", name="raw")
                        nc.vector.tensor_scalar_add(
                            out=raw, in0=ps, scalar1=bqk_sb[:, jt : jt + 1]
                        )
                        # RoPE: out = raw*cos2 + swap_halves(raw)*sin2
                        # half-swap on PE via permutation matmul (DVE
                        # can't move data across partitions)
                        dest = dest_all[:, h, ts(th, 512)]
                        ps_swp = ps_s_pool.tile(
                            [128, 512], f32, tag="ps_sc", name="ps_swp"
                        )
                        nc.tensor.matmul(
                            ps_swp, lhsT=perm_sb, rhs=raw, start=True, stop=True
                        )
                        rtmp = work.tile([128, 512], bf16, tag="rtmp", name="rtmp")
                        nc.vector.tensor_mul(rtmp, ps_swp, sin_sb[:, ts(th, 512)])
                        nc.vector.tensor_mul(dest, raw, cos_sb[:, ts(th, 512)])
                        nc.vector.tensor_add(dest, dest, rtmp)
            else:  # V, natural layout [t, j]
                jc = chunk - 4  # 0 or 1
                for tt in range(8):
                    ps = pspool.tile([128, 512], f32, tag="ps", name="ps")
                    for ct in range(16):
                        nc.tensor.matmul(
                            ps,
                            lhsT=xs[:, ct, ts(tt, 128)],
                            rhs=wt[:, ct, :],
                            start=(ct == 0),
                            stop=(ct == 15),
                        )
                    nc.vector.tensor_add(
                        v_all[:, tt, jc * 512 : (jc + 1) * 512],
                        ps,
                        bv_bc[:, jc * 512 : (jc + 1) * 512],
                    )

    # proj weights: load now so the DMA overlaps phase B
    projpool = tc.tile_pool(name=f"proj{rep}", bufs=1)
    proj = projpool.__enter__()
    try:
        wp = proj.tile([128, 16, C // 2], bf16, name="wp")
        nc.sync.dma_start(out=wp, in_=wproj_r)

        # =========== Phase B: attention per (head, q-chunk) ===========
        for h in range(HG):
            for qc in range(2):  # q chunks of 512
                n_kt = 4 * (qc + 1)  # causal: valid k tiles
                ps_y = ps_y_pool.tile([128, 512], f32, tag="ps_y", name="ps_y")
                ps_sum = ps_sum_pool.tile(
                    [1, 512], f32, tag="ps_sum", name="ps_sum"
                )
                for kt in range(n_kt):
                    ps_sc = ps_s_pool.tile(
                        [128, 512], f32, tag="ps_sc", name="ps_sc"
                    )
                    nc.tensor.matmul(
                        ps_sc,
                        lhsT=kf[:, h, ts(kt, 128)],
                        rhs=qf[:, h, ts(qc, 512)],
                        start=True,
                        stop=True,
                    )
                    p_sb = work.tile(
                        [128, 512], bf16, tag="p_sb", name="p_sb",
                        bufs=TUNE["p_sb_bufs"],
                    )
                    nc.scalar.activation(p_sb, ps_sc, AF.Exp, scale=scale)
                    kt_rel = kt - 4 * qc
                    if 0 <= kt_rel < 4:  # block straddles the diagonal
                        nc.vector.tensor_mul(p_sb, p_sb, mask_sb[:, kt_rel, :])
                    nc.tensor.matmul(
                        ps_sum,
                        lhsT=ones_sb,
                        rhs=p_sb,
                        start=(kt == 0),
                        stop=(kt == n_kt - 1),
                    )
                    nc.tensor.matmul(
                        ps_y,
                        lhsT=v_all[:, kt, ts(h, 128)],
                        rhs=p_sb,
                        start=(kt == 0),
                        stop=(kt == n_kt - 1),
                    )
                recip = work.tile([1, 512], f32, tag="recip", name="recip")
                nc.vector.reciprocal(recip, ps_sum)
                # broadcast across partitions via a DRAM bounce (SBUF
                # source DMAs can't have partition step 0; DRAM can)
                rdram = drampool.tile(
                    [1, 512], f32, tag="rdram", name="rdram", bufs=4
                )
                nc.sync.dma_start(out=rdram, in_=recip)
                rb = work.tile([128, 512], f32, tag="recip_bc", name="rb")
                nc.sync.dma_start(out=rb, in_=rdram.to_broadcast([128, 512]))
                nc.vector.tensor_mul(yT[:, h, ts(qc, 512)], ps_y, rb)

        # =========== Phase C: AllGather + c_proj ===========
        ybounce = drampool.tile([HG * D, T], bf16, name="ybounce")
        ygth = drampool.tile([2 * HG * D, T], bf16, name="ygth")
        yb_r = ybounce.rearrange("(h p) t -> p h t", p=128)
        for h in range(HG):
            nc.sync.dma_start(out=yb_r[:, h, :], in_=yT[:, h, :])
        if collective:
            nc.gpsimd.collective_compute(
                "AllGather",
                mybir.AluOpType.bypass,
                replica_groups=[[0, 1], [2, 3], [4, 5], [6, 7]],
                ins=[ybounce[:].opt()],
                outs=[ygth[:].opt()],
            )
        else:  # timeline-sim variant: fake the gather with local copies
            nc.sync.dma_start(out=ygth[0 : HG * D, :], in_=ybounce[:])
            nc.sync.dma_start(out=ygth[HG * D :, :], in_=ybounce[:])

        ygs = proj.tile([128, 16, T], bf16, name="ygs")
        nc.sync.dma_start(out=ygs, in_=ygth.rearrange("(jt p) t -> p jt t", p=128))
        for tt in range(8):
            for cc in range(2):  # output col chunks of 512
                ps = pspool.tile([128, 512], f32, tag="ps", name="ps_proj")
                for jt in range(16):
                    nc.tensor.matmul(
                        ps,
                        lhsT=ygs[:, jt, ts(tt, 128)],
                        rhs=wp[:, jt, ts(cc, 512)],
                        start=(jt == 0),
                        stop=(jt == 15),
                    )
                o_sb = work.tile([128, 512], f32, tag="o_sb", name="o_sb")
                nc.vector.tensor_add(o_sb, ps, bp_bc[:, ts(cc, 512)])
                nc.sync.dma_start(
                    out=out.ap()[ts(tt, 128), ts(cc, 512)], in_=o_sb
                )
    finally:
        projpool.__exit__(None, None, None)


def _host_inputs(x, w_attn, b_attn, w_proj, b_proj):
    """Build the 8 per-core input maps."""
    x = np.asarray(x, np.float32)
    w_attn = np.asarray(w_attn, np.float32)
    b_attn = np.asarray(b_attn, np.float32)
    w_proj = np.asarray(w_proj, np.float32)
    b_proj = np.asarray(b_proj, np.float32)

    # rope tables, transposed [d, t], full height with rotate-half signs folded:
    # out = x * cos2 + swap_halves(x) * sin2,  cos2=[cos;cos], sin2=[-sin;sin]
    inv_freq = 1.0 / (ROPE_BASE ** (np.arange(0, D, 2, dtype=np.float32) / D))
    freqs = np.arange(T, dtype=np.float32)[:, None] * inv_freq[None, :]  # [T, 64]
    c_ = np.ascontiguousarray(np.cos(freqs).T)  # [64, T]
    s_ = np.ascontiguousarray(np.sin(freqs).T)
    cosT = np.concatenate([c_, c_], axis=0).astype(BF16)  # [128, T]
    sinT = np.concatenate([-s_, s_], axis=0).astype(BF16)

    # causal mask blocks, transposed [k, q]: block kt_rel r, q chunk of 512
    k_idx = np.arange(128)
    q_idx = np.arange(512)
    maskT = np.zeros((128, 4, 512), np.float32)
    for r in range(4):
        maskT[:, r, :] = ((r * 128 + k_idx)[:, None] <= q_idx[None, :]).astype(
            np.float32
        )
    maskT = maskT.astype(BF16)

    permM = np.zeros((128, 128), np.float32)
    permM[(np.arange(128) + 64) % 128, np.arange(128)] = 1.0
    permM = permM.astype(BF16)

    in_maps = []
    for c in range(N_CORES):
        b, g = divmod(c, 2)
        cs = slice(g * 1024, (g + 1) * 1024)
        wq = w_attn[:, 0:C][:, cs]
        wk = w_attn[:, C : 2 * C][:, cs]
        wv = w_attn[:, 2 * C : 3 * C][:, cs]
        bq = b_attn[0:C][cs]
        bk = b_attn[C : 2 * C][cs]
        bvv = b_attn[2 * C : 3 * C][cs]
        in_maps.append(
            {
                "xT": np.ascontiguousarray(x[b].T).astype(BF16),
                "wqkv": np.concatenate([wq, wk, wv], axis=1).astype(BF16),
                "bqk": np.ascontiguousarray(
                    np.concatenate([bq, bk]).reshape(16, 128).T
                ).astype(np.float32),
                "bv": bvv.reshape(1, 1024).astype(np.float32),
                "cosT": cosT,
                "sinT": sinT,
                "maskT": maskT,
                "perm": permM,
                "wproj": w_proj[:, cs].astype(BF16),
                "bproj": b_proj[cs].reshape(1, 1024).astype(np.float32),
            }
        )
    return in_maps


def kernel(x, w_attn, b_attn, w_proj, b_proj, _trace=False):
    from concourse.bass_utils import run_bass_kernel_spmd

    if "nc" not in _PROGRAM_CACHE:
        _PROGRAM_CACHE["nc"] = _build_program()
    nc = _PROGRAM_CACHE["nc"]

    in_maps = _host_inputs(x, w_attn, b_attn, w_proj, b_proj)
    res = run_bass_kernel_spmd(
        nc, in_maps, core_ids=list(range(N_CORES)), trace=_trace
    )
    _PROGRAM_CACHE["last_results"] = res

    out = np.zeros((B, T, C), np.float32)
    for c in range(N_CORES):
        b, g = divmod(c, 2)
        out[b, :, g * 1024 : (g + 1) * 1024] = res.results[c]["out"]
    return out



# revision 3
# speedup vs baseline: 1.0726x; 1.0726x over previous
"""Causal self-attention (B=4, T=1024, C=2048, H=16, rotary) on 8 trn2 cores.

Sharding: core c = 2*b + g handles batch b, head-group g (heads 8g..8g+7).
 - QKV projection in transposed layout (Q^T/K^T = [d, T]; V natural [T, d]).
 - RoPE via full-height cos/sin tables; rotate-half partition swap runs on
   the PE as a permutation matmul.
 - Scores transposed S^T = K^T.Q -> [k, q]; softmax without max-subtraction;
   causal masking via multiplicative 0/1 bf16 masks on diagonal blocks.
 - Softmax denominator: ones[128,128] matmul accumulates column sums already
   broadcast across partitions; 1/d = exp(-ln d) on the scalar engine (Ln и
   Exp share one ACT table set, so no table reloads).
 - att@V accumulated in PSUM -> y^T [d, q], normalized by rb while copying
   PSUM->SBUF.
 - Emission interleaves phases so the Tile scheduler overlaps them:
   chunks(q0,k0,v0) -> attn h0-3 -> AllGather(h0-3) -> chunks(q1,k1,v1)
   -> attn h4-7 -> AllGather(h4-7) -> c_proj waveA (gathered heads 0-3 both
   cores, + bias -> stash) -> waveB (remaining heads) + merge -> out.
All matmuls bf16 (fp32 PSUM accumulation).
"""

import math

import numpy as np
import ml_dtypes

BF16 = ml_dtypes.bfloat16

B, T, C = 4, 1024, 2048
H = 16  # total heads
D = C // H  # 128 head dim
HG = 8  # heads per group (per core)
N_CORES = 8
ROPE_BASE = 10000.0

TUNE = {
    "ps_a": 2,
    "ps_b": 3,
    "ps_y": 2,
    "p_sb_bufs": 6,
}

_PROGRAM_CACHE = {}


def _build_program(num_devices=N_CORES, collective=True):
    import concourse.mybir as mybir
    import concourse.tile as tile
    from concourse import bacc
    from concourse.bass import ts

    f32 = mybir.dt.float32
    bf16 = mybir.dt.bfloat16
    AF = mybir.ActivationFunctionType

    nc = bacc.Bacc(trn_type="TRN2", num_devices=num_devices, debug=False)

    # ---- per-core I/O ----
    xT = nc.dram_tensor("xT", [C, T], bf16, kind="ExternalInput")  # x[b].T
    wqkv = nc.dram_tensor("wqkv", [C, 3 * HG * D], bf16, kind="ExternalInput")
    bqk = nc.dram_tensor("bqk", [128, 16], f32, kind="ExternalInput")
    bv = nc.dram_tensor("bv", [1, HG * D], f32, kind="ExternalInput")
    # full-height rope tables: cos2 = [cos; cos], sin2 = [-sin; sin]
    cosT = nc.dram_tensor("cosT", [D, T], bf16, kind="ExternalInput")
    sinT = nc.dram_tensor("sinT", [D, T], bf16, kind="ExternalInput")
    maskT = nc.dram_tensor("maskT", [128, 4, 512], bf16, kind="ExternalInput")
    # half-swap permutation: perm[j2, j] = 1 iff j2 == (j + 64) % 128
    perm = nc.dram_tensor("perm", [128, 128], bf16, kind="ExternalInput")
    wproj = nc.dram_tensor("wproj", [C, C // 2], bf16, kind="ExternalInput")
    bproj = nc.dram_tensor("bproj", [1, C // 2], f32, kind="ExternalInput")
    out = nc.dram_tensor("out", [T, C // 2], f32, kind="ExternalOutput")

    xT_r = xT.ap().rearrange("(ct p) t -> p ct t", p=128)  # [128, 16, 1024]
    wqkv_r = wqkv.ap().rearrange("(ct p) j -> p ct j", p=128)  # [128, 16, 3072]
    wproj_r = wproj.ap().rearrange("(jt p) c -> p jt c", p=128)  # [128, 16, 1024]

    scale = 1.0 / math.sqrt(D)

    with tile.TileContext(nc) as tc:
        with (
            tc.tile_pool(name="const", bufs=1) as const,
            tc.tile_pool(name="persist", bufs=1) as persist,
            tc.tile_pool(name="wp_pool", bufs=1) as wp_pool,
            tc.tile_pool(name="ps_a", bufs=TUNE["ps_a"], space="PSUM") as psA,
            tc.tile_pool(name="ps_b", bufs=TUNE["ps_b"], space="PSUM") as psB,
            tc.tile_pool(name="ps_y", bufs=TUNE["ps_y"], space="PSUM") as psY,
            tc.tile_pool(name="ps_sum", bufs=1, space="PSUM") as psS,
            tc.tile_pool(name="work", bufs=4) as work,
            tc.tile_pool(name="dram", bufs=1, space="DRAM") as drampool,
        ):
            # ---- persistent activations ----
            qf = persist.tile([128, HG, T], bf16)  # [d, h, t] rotated Q^T
            kf = persist.tile([128, HG, T], bf16)  # [d, h, t] rotated K^T
            v_all = persist.tile([128, 8, HG * D], bf16)  # [t_in, tt, j]
            yT = persist.tile([128, HG, T], bf16)  # [d, h, t] normalized att out

            # ---- DRAM staging for the collective ----
            ybounce = drampool.tile([HG * D, T], bf16, name="ybounce")
            ygthA = drampool.tile([HG * D, T], bf16, name="ygthA")
            ygthB = drampool.tile([HG * D, T], bf16, name="ygthB")
            yb_r = ybounce.rearrange("(h p) t -> p h t", p=128)

            with (
                tc.tile_pool(name="xpool", bufs=1) as xpool,
                tc.tile_pool(name="wpool", bufs=2) as wpool,
            ):
                xs = xpool.tile([128, 16, T], bf16, name="xs")
                wts = {}

                def load_chunk(chunk, interleave_xs=False):
                    wt = wpool.tile([128, 16, 512], bf16, tag="wt", name="wt")
                    wts[chunk] = wt
                    for ct in range(16):
                        if interleave_xs:
                            nc.sync.dma_start(out=xs[:, ct, :], in_=xT_r[:, ct, :])
                        nc.sync.dma_start(
                            out=wt[:, ct, :],
                            in_=wqkv_r[:, ct, chunk * 512 : (chunk + 1) * 512],
                        )

                # first chunk's weights interleaved with xs so matmuls start
                # within ~2us of kernel start
                load_chunk(0, interleave_xs=True)

                # ---- constants (after the critical first-chunk DMAs) ----
                bqk_sb = const.tile([128, 16], f32)
                nc.sync.dma_start(out=bqk_sb, in_=bqk.ap())
                cos_sb = const.tile([128, T], bf16)
                nc.sync.dma_start(out=cos_sb, in_=cosT.ap())
                sin_sb = const.tile([128, T], bf16)
                nc.sync.dma_start(out=sin_sb, in_=sinT.ap())
                perm_sb = const.tile([128, 128], bf16)
                nc.sync.dma_start(out=perm_sb, in_=perm.ap())
                ones128 = const.tile([128, 128], bf16)
                nc.vector.memset(ones128, 1.0)
                mask_sb = const.tile([128, 4, 512], bf16)
                nc.sync.dma_start(out=mask_sb, in_=maskT.ap())
                bv_bc = const.tile([128, HG * D], f32)
                nc.sync.dma_start(out=bv_bc, in_=bv.ap().to_broadcast([128, HG * D]))
                bp_bc = const.tile([128, C // 2], f32)
                nc.sync.dma_start(out=bp_bc, in_=bproj.ap().to_broadcast([128, C // 2]))

                wp = wp_pool.tile([128, 16, C // 2], bf16, name="wp")

                def emit_chunk(chunk):
                    wt = wts[chunk]
                    if chunk < 4:  # Q or K, output transposed [j, t]
                        for jj in range(4):
                            jt = chunk * 4 + jj  # q: 0-7, k: 8-15
                            h = jt % 8
                            dest_all = qf if jt < 8 else kf
                            for th in range(2):
                                ps = psA.tile([128, 512], f32, tag="ps", name="ps")
                                for ct in range(16):
                                    nc.tensor.matmul(
                                        ps,
                                        lhsT=wt[:, ct, jj * 128 : (jj + 1) * 128],
                                        rhs=xs[:, ct, ts(th, 512)],
                                        start=(ct == 0),
                                        stop=(ct == 15),
                                    )
                                raw = work.tile(
                                    [128, 512], bf16, tag="raw", name="raw",
                                    bufs=3,
                                )
                                # bias-add on ACT (Identity supports AP bias);
                                # keeps DVE free for the rope muls
                                nc.scalar.activation(
                                    raw, ps, AF.Identity,
                                    bias=bqk_sb[:, jt : jt + 1],
                                )
                                dest = dest_all[:, h, ts(th, 512)]
                                ps_swp = psB.tile(
                                    [128, 512], f32, tag="psb", name="ps_swp"
                                )
                                nc.tensor.matmul(
                                    ps_swp, lhsT=perm_sb, rhs=raw,
                                    start=True, stop=True,
                                )
                                rtmp = work.tile(
                                    [128, 512], bf16, tag="rtmp", name="rtmp",
                                    bufs=3,
                                )
                                nc.vector.tensor_mul(
                                    rtmp, ps_swp, sin_sb[:, ts(th, 512)]
                                )
                                nc.vector.tensor_mul(
                                    dest, raw, cos_sb[:, ts(th, 512)]
                                )
                                nc.vector.tensor_add(dest, dest, rtmp)
                    else:  # V, natural layout [t, j]
                        jc = chunk - 4  # 0 or 1
                        for tt in range(8):
                            ps = psA.tile([128, 512], f32, tag="ps", name="ps")
                            for ct in range(16):
                                nc.tensor.matmul(
                                    ps,
                                    lhsT=xs[:, ct, ts(tt, 128)],
                                    rhs=wt[:, ct, :],
                                    start=(ct == 0),
                                    stop=(ct == 15),
                                )
                            nc.vector.tensor_add(
                                v_all[:, tt, jc * 512 : (jc + 1) * 512],
                                ps,
                                bv_bc[:, jc * 512 : (jc + 1) * 512],
                            )

                def emit_attn(h):
                    for qc in range(2):
                        n_kt = 4 * (qc + 1)
                        ps_y = psY.tile([128, 512], f32, tag="ps_y", name="ps_y")
                        ps_sum = psS.tile(
                            [128, 512], f32, tag="ps_sum", name="ps_sum"
                        )
                        p_hold = None
                        for kt in range(n_kt):
                            ps_sc = psB.tile(
                                [128, 512], f32, tag="psb", name="ps_sc"
                            )
                            nc.tensor.matmul(
                                ps_sc,
                                lhsT=kf[:, h, ts(kt, 128)],
                                rhs=qf[:, h, ts(qc, 512)],
                                start=True,
                                stop=True,
                            )
                            p_sb = work.tile(
                                [128, 512], bf16, tag="p_sb", name="p_sb",
                                bufs=TUNE["p_sb_bufs"],
                            )
                            nc.scalar.activation(p_sb, ps_sc, AF.Exp, scale=scale)
                            kt_rel = kt - 4 * qc
                            if 0 <= kt_rel < 4:  # block straddles the diagonal
                                nc.vector.tensor_mul(
                                    p_sb, p_sb, mask_sb[:, kt_rel, :]
                                )
                            if kt % 2 == 0:
                                p_hold = p_sb
                            else:
                                padd = work.tile(
                                    [128, 512], bf16, tag="padd", name="padd",
                                    bufs=2,
                                )
                                nc.vector.tensor_add(padd, p_hold, p_sb)
                                # ones[128,128] stationary: the column sums land
                                # broadcast across all 128 partitions
                                nc.tensor.matmul(
                                    ps_sum,
                                    lhsT=ones128,
                                    rhs=padd,
                                    start=(kt == 1),
                                    stop=(kt == n_kt - 1),
                                )
                            nc.tensor.matmul(
                                ps_y,
                                lhsT=v_all[:, kt, ts(h, 128)],
                                rhs=p_sb,
                                start=(kt == 0),
                                stop=(kt == n_kt - 1),
                            )
                        # 1/denom = exp(-ln(denom)); Ln/Exp share a table set
                        lnt = work.tile(
                            [128, 512], f32, tag="lnt", name="lnt", bufs=2
                        )
                        nc.scalar.activation(lnt, ps_sum, AF.Ln)
                        rb = work.tile(
                            [128, 512], bf16, tag="rb", name="rb", bufs=2
                        )
                        nc.scalar.activation(rb, lnt, AF.Exp, scale=-1.0)
                        nc.vector.tensor_mul(yT[:, h, ts(qc, 512)], ps_y, rb)
                    nc.sync.dma_start(out=yb_r[:, h, :], in_=yT[:, h, :])

                # ========== interleaved emission ==========
                for chunk in (2, 4):
                    load_chunk(chunk)
                for chunk in (0, 2, 4):
                    emit_chunk(chunk)
                for h in range(4):
                    emit_attn(h)
                if collective:
                    nc.gpsimd.collective_compute(
                        "AllGather",
                        mybir.AluOpType.bypass,
                        replica_groups=[[0, 1], [2, 3], [4, 5], [6, 7]],
                        ins=[ybounce[0 : HG * D // 2, :].opt()],
                        outs=[ygthA[:].opt()],
                    )
                else:
                    nc.sync.dma_start(
                        out=ygthA[0 : HG * D // 2, :],
                        in_=ybounce[0 : HG * D // 2, :],
                    )
                    nc.sync.dma_start(
                        out=ygthA[HG * D // 2 :, :],
                        in_=ybounce[0 : HG * D // 2, :],
                    )
                for chunk in (1, 3, 5):
                    load_chunk(chunk)
                # proj weights: DMA issues behind the chunk loads, done long
                # before waveA needs it
                nc.sync.dma_start(out=wp, in_=wproj_r)
                for chunk in (1, 3, 5):
                    emit_chunk(chunk)
                for h in range(4, 8):
                    emit_attn(h)
                if collective:
                    nc.gpsimd.collective_compute(
                        "AllGather",
                        mybir.AluOpType.bypass,
                        replica_groups=[[0, 1], [2, 3], [4, 5], [6, 7]],
                        ins=[ybounce[HG * D // 2 :, :].opt()],
                        outs=[ygthB[:].opt()],
                    )
                else:
                    nc.sync.dma_start(
                        out=ygthB[0 : HG * D // 2, :],
                        in_=ybounce[HG * D // 2 :, :],
                    )
                    nc.sync.dma_start(
                        out=ygthB[HG * D // 2 :, :],
                        in_=ybounce[HG * D // 2 :, :],
                    )

            # ========== c_proj: two waves over the gathered halves ==========
            # ygthA rows = global heads 0-3 (rank0) + 8-11 (rank1);
            # ygthB rows = global heads 4-7 + 12-15.
            with (
                tc.tile_pool(name="stash_pool", bufs=1) as stash_pool,
                tc.tile_pool(name="ygs_pool", bufs=1) as ygs_pool,
            ):
                stash = stash_pool.tile([128, 16, 512], f32, name="stash")
                jtA = (0, 1, 2, 3, 8, 9, 10, 11)
                jtB = (4, 5, 6, 7, 12, 13, 14, 15)
                for which, (ygth, jts) in enumerate(
                    ((ygthA, jtA), (ygthB, jtB))
                ):
                    ygs = ygs_pool.tile(
                        [128, 8, T], bf16, tag=f"ygs{which}", name=f"ygs{which}"
                    )
                    nc.sync.dma_start(
                        out=ygs, in_=ygth.rearrange("(j p) t -> p j t", p=128)
                    )
                    for tt in range(8):
                        for cc in range(2):
                            st = tt * 2 + cc
                            ps = psA.tile(
                                [128, 512], f32, tag="ps", name="ps_proj"
                            )
                            for i, jt in enumerate(jts):
                                nc.tensor.matmul(
                                    ps,
                                    lhsT=ygs[:, i, ts(tt, 128)],
                                    rhs=wp[:, jt, ts(cc, 512)],
                                    start=(i == 0),
                                    stop=(i == 7),
                                )
                            if which == 0:
                                nc.vector.tensor_add(
                                    stash[:, st, :], ps, bp_bc[:, ts(cc, 512)]
                                )
                            else:
                                o_sb = work.tile(
                                    [128, 512], f32, tag="o_sb", name="o_sb",
                                    bufs=3,
                                )
                                nc.vector.tensor_add(o_sb, ps, stash[:, st, :])
                                nc.sync.dma_start(
                                    out=out.ap()[ts(tt, 128), ts(cc, 512)],
                                    in_=o_sb,
                                )

    nc.finalize()
    return nc


def _host_inputs(x, w_attn, b_attn, w_proj, b_proj):
    """Build the 8 per-core input maps."""
    x = np.asarray(x, np.float32)
    w_attn = np.asarray(w_attn, np.float32)
    b_attn = np.asarray(b_attn, np.float32)
    w_proj = np.asarray(w_proj, np.float32)
    b_proj = np.asarray(b_proj, np.float32)

    # rope tables, transposed [d, t], full height with rotate-half signs folded:
    # out = x * cos2 + swap_halves(x) * sin2,  cos2=[cos;cos], sin2=[-sin;sin]
    inv_freq = 1.0 / (ROPE_BASE ** (np.arange(0, D, 2, dtype=np.float32) / D))
    freqs = np.arange(T, dtype=np.float32)[:, None] * inv_freq[None, :]  # [T, 64]
    c_ = np.ascontiguousarray(np.cos(freqs).T)  # [64, T]
    s_ = np.ascontiguousarray(np.sin(freqs).T)
    cosT = np.concatenate([c_, c_], axis=0).astype(BF16)  # [128, T]
    sinT = np.concatenate([-s_, s_], axis=0).astype(BF16)

    # causal mask blocks, transposed [k, q]: block kt_rel r, q chunk of 512
    k_idx = np.arange(128)
    q_idx = np.arange(512)
    maskT = np.zeros((128, 4, 512), np.float32)
    for r in range(4):
        maskT[:, r, :] = ((r * 128 + k_idx)[:, None] <= q_idx[None, :]).astype(
            np.float32
        )
    maskT = maskT.astype(BF16)

    permM = np.zeros((128, 128), np.float32)
    permM[(np.arange(128) + 64) % 128, np.arange(128)] = 1.0
    permM = permM.astype(BF16)

    in_maps = []
    for c in range(N_CORES):
        b, g = divmod(c, 2)
        cs = slice(g * 1024, (g + 1) * 1024)
        wq = w_attn[:, 0:C][:, cs]
        wk = w_attn[:, C : 2 * C][:, cs]
        wv = w_attn[:, 2 * C : 3 * C][:, cs]
        bq = b_attn[0:C][cs]
        bk = b_attn[C : 2 * C][cs]
        bvv = b_attn[2 * C : 3 * C][cs]
        in_maps.append(
            {
                "xT": np.ascontiguousarray(x[b].T).astype(BF16),
                "wqkv": np.concatenate([wq, wk, wv], axis=1).astype(BF16),
                "bqk": np.ascontiguousarray(
                    np.concatenate([bq, bk]).reshape(16, 128).T
                ).astype(np.float32),
                "bv": bvv.reshape(1, 1024).astype(np.float32),
                "cosT": cosT,
                "sinT": sinT,
                "maskT": maskT,
                "perm": permM,
                "wproj": w_proj[:, cs].astype(BF16),
                "bproj": b_proj[cs].reshape(1, 1024).astype(np.float32),
            }
        )
    return in_maps


def kernel(x, w_attn, b_attn, w_proj, b_proj, _trace=False):
    from concourse.bass_utils import run_bass_kernel_spmd

    if "nc" not in _PROGRAM_CACHE:
        _PROGRAM_CACHE["nc"] = _build_program()
    nc = _PROGRAM_CACHE["nc"]

    in_maps = _host_inputs(x, w_attn, b_attn, w_proj, b_proj)
    res = run_bass_kernel_spmd(
        nc, in_maps, core_ids=list(range(N_CORES)), trace=_trace
    )
    _PROGRAM_CACHE["last_results"] = res

    out = np.zeros((B, T, C), np.float32)
    for c in range(N_CORES):
        b, g = divmod(c, 2)
        out[b, :, g * 1024 : (g + 1) * 1024] = res.results[c]["out"]
    return out


# revision 6
# speedup vs baseline: 1.4330x; 1.3359x over previous
"""Causal self-attention (B=4, T=1024, C=2048, H=16, rotary) on 8 trn2 cores.

Sharding: core c = 2*b + g handles batch b, head-group g (heads 8g..8g+7).
 - QKV projection in transposed layout (Q^T/K^T = [d, T]; V natural [T, d]).
 - RoPE via full-height cos/sin tables; rotate-half partition swap runs on
   the PE as a permutation matmul.
 - Scores transposed S^T = K^T.Q -> [k, q]; softmax without max-subtraction;
   causal masking via multiplicative 0/1 bf16 masks on diagonal blocks.
 - Softmax denominator: ones[128,128] matmul accumulates column sums already
   broadcast across partitions; 1/d = exp(-ln d) on the scalar engine (Ln и
   Exp share one ACT table set, so no table reloads).
 - att@V accumulated in PSUM -> y^T [d, q], normalized by rb while copying
   PSUM->SBUF.
 - Emission interleaves phases so the Tile scheduler overlaps them:
   chunks(q0,k0,v0) -> attn h0-3 -> AllGather(h0-3) -> chunks(q1,k1,v1)
   -> attn h4-7 -> AllGather(h4-7) -> c_proj waveA (gathered heads 0-3 both
   cores, + bias -> stash) -> waveB (remaining heads) + merge -> out.
All matmuls bf16 (fp32 PSUM accumulation).
"""

import math

import numpy as np
import ml_dtypes

BF16 = ml_dtypes.bfloat16

B, T, C = 4, 1024, 2048
H = 16  # total heads
D = C // H  # 128 head dim
HG = 8  # heads per group (per core)
N_CORES = 8
ROPE_BASE = 10000.0

TUNE = {
    "ps_a": 2,
    "ps_b": 3,
    "ps_y": 2,
    "p_sb_bufs": 6,
}

_PROGRAM_CACHE = {}


def _build_program(num_devices=N_CORES, collective=True):
    import concourse.mybir as mybir
    import concourse.tile as tile
    from concourse import bacc
    from concourse.bass import ts

    f32 = mybir.dt.float32
    bf16 = mybir.dt.bfloat16
    AF = mybir.ActivationFunctionType

    nc = bacc.Bacc(trn_type="TRN2", num_devices=num_devices, debug=False)

    # ---- per-core I/O ----
    xT = nc.dram_tensor("xT", [C, T], bf16, kind="ExternalInput")  # x[b].T
    wqkv = nc.dram_tensor("wqkv", [C, 3 * HG * D], bf16, kind="ExternalInput")
    bqk = nc.dram_tensor("bqk", [128, 16], f32, kind="ExternalInput")
    bv = nc.dram_tensor("bv", [1, HG * D], f32, kind="ExternalInput")
    # full-height rope tables: cos2 = [cos; cos], sin2 = [-sin; sin]
    cosT = nc.dram_tensor("cosT", [D, T], bf16, kind="ExternalInput")
    sinT = nc.dram_tensor("sinT", [D, T], bf16, kind="ExternalInput")
    maskT = nc.dram_tensor("maskT", [128, 4, 512], bf16, kind="ExternalInput")
    # half-swap permutation: perm[j2, j] = 1 iff j2 == (j + 64) % 128
    perm = nc.dram_tensor("perm", [128, 128], bf16, kind="ExternalInput")
    wproj = nc.dram_tensor("wproj", [C, C // 2], bf16, kind="ExternalInput")
    bproj = nc.dram_tensor("bproj", [1, C // 2], f32, kind="ExternalInput")
    out = nc.dram_tensor("out", [T, C // 2], f32, kind="ExternalOutput")

    xT_r = xT.ap().rearrange("(ct p) t -> p ct t", p=128)  # [128, 16, 1024]
    wqkv_r = wqkv.ap().rearrange("(ct p) j -> p ct j", p=128)  # [128, 16, 3072]
    wproj_r = wproj.ap().rearrange("(jt p) c -> p jt c", p=128)  # [128, 16, 1024]

    scale = 1.0 / math.sqrt(D)

    with tile.TileContext(nc) as tc:
        with (
            tc.tile_pool(name="const", bufs=1) as const,
            tc.tile_pool(name="persist", bufs=1) as persist,
            tc.tile_pool(name="wp_pool", bufs=1) as wp_pool,
            tc.tile_pool(name="ps_a", bufs=TUNE["ps_a"], space="PSUM") as psA,
            tc.tile_pool(name="ps_b", bufs=TUNE["ps_b"], space="PSUM") as psB,
            tc.tile_pool(name="ps_y", bufs=TUNE["ps_y"], space="PSUM") as psY,
            tc.tile_pool(name="ps_sum", bufs=1, space="PSUM") as psS,
            tc.tile_pool(name="work", bufs=4) as work,
            tc.tile_pool(name="dram", bufs=1, space="DRAM") as drampool,
        ):
            # ---- persistent activations ----
            qf = persist.tile([128, HG, T], bf16)  # [d, h, t] rotated Q^T
            kf = persist.tile([128, HG, T], bf16)  # [d, h, t] rotated K^T
            v_all = persist.tile([128, 8, HG * D], bf16)  # [t_in, tt, j]
            yT = persist.tile([128, HG, T], bf16)  # [d, h, t] normalized att out

            # ---- DRAM staging for the collective (4 quarters) ----
            ybounce = drampool.tile([HG * D, T], bf16, name="ybounce")
            ygth_q = [
                drampool.tile([2 * 2 * D, T], bf16, name=f"ygth{w}")
                for w in range(4)
            ]
            yb_r = ybounce.rearrange("(h p) t -> p h t", p=128)

            with (
                tc.tile_pool(name="xpool", bufs=1) as xpool,
                tc.tile_pool(name="wpool", bufs=2) as wpool,
            ):
                xs = xpool.tile([128, 16, T], bf16, name="xs")
                wts = {}

                def load_chunk(chunk, interleave_xs=False):
                    wt = wpool.tile([128, 16, 512], bf16, tag="wt", name="wt")
                    wts[chunk] = wt
                    cslice = slice(chunk * 512, (chunk + 1) * 512)
                    if interleave_xs:
                        # 4-ct granules: first matmul chain starts after ~1/4
                        # of the data, and DMA-issue count stays low
                        for q in range(4):
                            cts = slice(4 * q, 4 * q + 4)
                            nc.sync.dma_start(out=xs[:, cts, :], in_=xT_r[:, cts, :])
                            nc.sync.dma_start(
                                out=wt[:, cts, :], in_=wqkv_r[:, cts, cslice]
                            )
                    else:
                        nc.sync.dma_start(out=wt, in_=wqkv_r[:, :, cslice])

                # first chunk's weights interleaved with xs so matmuls start
                # within ~2us of kernel start
                load_chunk(0, interleave_xs=True)

                # ---- constants (after the critical first-chunk DMAs) ----
                bqk_sb = const.tile([128, 16], f32)
                nc.sync.dma_start(out=bqk_sb, in_=bqk.ap())
                cos_sb = const.tile([128, T], bf16)
                nc.sync.dma_start(out=cos_sb, in_=cosT.ap())
                sin_sb = const.tile([128, T], bf16)
                nc.sync.dma_start(out=sin_sb, in_=sinT.ap())
                perm_sb = const.tile([128, 128], bf16)
                nc.sync.dma_start(out=perm_sb, in_=perm.ap())
                ones128 = const.tile([128, 128], bf16)
                nc.vector.memset(ones128, 1.0)
                mask_sb = const.tile([128, 4, 512], bf16)
                nc.sync.dma_start(out=mask_sb, in_=maskT.ap())
                bv_bc = const.tile([128, HG * D], f32)
                nc.sync.dma_start(out=bv_bc, in_=bv.ap().to_broadcast([128, HG * D]))
                bp_bc = const.tile([128, C // 2], f32)
                nc.sync.dma_start(out=bp_bc, in_=bproj.ap().to_broadcast([128, C // 2]))

                wp = wp_pool.tile([128, 16, C // 2], bf16, name="wp")

                def emit_chunk(chunk):
                    wt = wts[chunk]
                    if chunk < 4:  # Q or K, output transposed [j, t]
                        for jj in range(4):
                            jt = chunk * 4 + jj  # q: 0-7, k: 8-15
                            h = jt % 8
                            dest_all = qf if jt < 8 else kf
                            for th in range(2):
                                ps = psA.tile([128, 512], f32, tag="ps", name="ps")
                                for ct in range(16):
                                    nc.tensor.matmul(
                                        ps,
                                        lhsT=wt[:, ct, jj * 128 : (jj + 1) * 128],
                                        rhs=xs[:, ct, ts(th, 512)],
                                        start=(ct == 0),
                                        stop=(ct == 15),
                                    )
                                raw = work.tile(
                                    [128, 512], bf16, tag="raw", name="raw",
                                    bufs=3,
                                )
                                # bias-add on ACT (Identity supports AP bias);
                                # keeps DVE free for the rope muls
                                nc.scalar.activation(
                                    raw, ps, AF.Identity,
                                    bias=bqk_sb[:, jt : jt + 1],
                                )
                                dest = dest_all[:, h, ts(th, 512)]
                                ps_swp = psB.tile(
                                    [128, 512], f32, tag="psb", name="ps_swp"
                                )
                                nc.tensor.matmul(
                                    ps_swp, lhsT=perm_sb, rhs=raw,
                                    start=True, stop=True,
                                )
                                rtmp = work.tile(
                                    [128, 512], bf16, tag="rtmp", name="rtmp",
                                    bufs=3,
                                )
                                nc.vector.tensor_mul(
                                    rtmp, ps_swp, sin_sb[:, ts(th, 512)]
                                )
                                nc.vector.tensor_mul(
                                    dest, raw, cos_sb[:, ts(th, 512)]
                                )
                                nc.vector.tensor_add(dest, dest, rtmp)
                    else:  # V, natural layout [t, j]
                        jc = chunk - 4  # 0 or 1
                        for tt in range(8):
                            ps = psA.tile([128, 512], f32, tag="ps", name="ps")
                            for ct in range(16):
                                nc.tensor.matmul(
                                    ps,
                                    lhsT=xs[:, ct, ts(tt, 128)],
                                    rhs=wt[:, ct, :],
                                    start=(ct == 0),
                                    stop=(ct == 15),
                                )
                            nc.vector.tensor_add(
                                v_all[:, tt, jc * 512 : (jc + 1) * 512],
                                ps,
                                bv_bc[:, jc * 512 : (jc + 1) * 512],
                            )

                def emit_attn(h):
                    for qc in range(2):
                        n_kt = 4 * (qc + 1)
                        ps_y = psY.tile([128, 512], f32, tag="ps_y", name="ps_y")
                        ps_sum = psS.tile(
                            [128, 512], f32, tag="ps_sum", name="ps_sum"
                        )
                        p_hold = None
                        for kt in range(n_kt):
                            ps_sc = psB.tile(
                                [128, 512], f32, tag="psb", name="ps_sc"
                            )
                            nc.tensor.matmul(
                                ps_sc,
                                lhsT=kf[:, h, ts(kt, 128)],
                                rhs=qf[:, h, ts(qc, 512)],
                                start=True,
                                stop=True,
                            )
                            p_sb = work.tile(
                                [128, 512], bf16, tag="p_sb", name="p_sb",
                                bufs=TUNE["p_sb_bufs"],
                            )
                            nc.scalar.activation(p_sb, ps_sc, AF.Exp, scale=scale)
                            kt_rel = kt - 4 * qc
                            if 0 <= kt_rel < 4:  # block straddles the diagonal
                                nc.vector.tensor_mul(
                                    p_sb, p_sb, mask_sb[:, kt_rel, :]
                                )
                            if kt % 2 == 0:
                                p_hold = p_sb
                            else:
                                padd = work.tile(
                                    [128, 512], bf16, tag="padd", name="padd",
                                    bufs=3,
                                )
                                nc.vector.tensor_add(padd, p_hold, p_sb)
                                if kt % 4 == 1:
                                    padd_hold = padd
                                else:
                                    pquad = work.tile(
                                        [128, 512], bf16, tag="pquad",
                                        name="pquad", bufs=2,
                                    )
                                    nc.vector.tensor_add(pquad, padd_hold, padd)
                                    # ones[128,128] stationary: column sums land
                                    # broadcast across all 128 partitions
                                    nc.tensor.matmul(
                                        ps_sum,
                                        lhsT=ones128,
                                        rhs=pquad,
                                        start=(kt == 3),
                                        stop=(kt == n_kt - 1),
                                    )
                            nc.tensor.matmul(
                                ps_y,
                                lhsT=v_all[:, kt, ts(h, 128)],
                                rhs=p_sb,
                                start=(kt == 0),
                                stop=(kt == n_kt - 1),
                            )
                        # 1/denom = exp(-ln(denom)); Ln/Exp share a table set
                        lnt = work.tile(
                            [128, 512], f32, tag="lnt", name="lnt", bufs=2
                        )
                        nc.scalar.activation(lnt, ps_sum, AF.Ln)
                        rb = work.tile(
                            [128, 512], bf16, tag="rb", name="rb", bufs=2
                        )
                        nc.scalar.activation(rb, lnt, AF.Exp, scale=-1.0)
                        nc.vector.tensor_mul(yT[:, h, ts(qc, 512)], ps_y, rb)
                    nc.sync.dma_start(out=yb_r[:, h, :], in_=yT[:, h, :])

                # ========== interleaved emission ==========
                def emit_ag(w):
                    rows = slice(2 * D * w, 2 * D * (w + 1))
                    if collective:
                        nc.gpsimd.collective_compute(
                            "AllGather",
                            mybir.AluOpType.bypass,
                            replica_groups=[[0, 1], [2, 3], [4, 5], [6, 7]],
                            ins=[ybounce[rows, :].opt()],
                            outs=[ygth_q[w][:].opt()],
                        )
                    else:
                        nc.sync.dma_start(
                            out=ygth_q[w][0 : 2 * D, :], in_=ybounce[rows, :]
                        )
                        nc.sync.dma_start(
                            out=ygth_q[w][2 * D :, :], in_=ybounce[rows, :]
                        )

                for chunk in (2, 4):
                    load_chunk(chunk)
                for chunk in (0, 2, 4):
                    emit_chunk(chunk)
                for h in range(4):
                    emit_attn(h)
                    if h % 2 == 1:
                        emit_ag(h // 2)
                for chunk in (1, 3, 5):
                    load_chunk(chunk)
                # proj weights: DMA issues behind the chunk loads, done long
                # before the first wave needs it
                nc.sync.dma_start(out=wp, in_=wproj_r)
                for chunk in (1, 3, 5):
                    emit_chunk(chunk)
                for h in range(4, 8):
                    emit_attn(h)
                    if h % 2 == 1:
                        emit_ag(h // 2)

            # ========== c_proj: four waves over the gathered quarters ======
            # ygth_q[w] rows = global heads (2w, 2w+1) from rank0 and
            # (8+2w, 8+2w+1) from rank1 -> global jt (2w, 2w+1, 8+2w, 9+2w).
            with (
                tc.tile_pool(name="stash_pool", bufs=1) as stash_pool,
                tc.tile_pool(name="ygs_pool", bufs=1) as ygs_pool,
            ):
                stash = stash_pool.tile([128, 16, 512], f32, name="stash")
                for w in range(4):
                    jts = (2 * w, 2 * w + 1, 8 + 2 * w, 9 + 2 * w)
                    ygs = ygs_pool.tile(
                        [128, 4, T], bf16, tag=f"ygs{w}", name=f"ygs{w}"
                    )
                    nc.sync.dma_start(
                        out=ygs,
                        in_=ygth_q[w].rearrange("(j p) t -> p j t", p=128),
                    )
                    for tt in range(8):
                        for cc in range(2):
                            st = tt * 2 + cc
                            ps = psA.tile(
                                [128, 512], f32, tag="ps", name="ps_proj"
                            )
                            for i, jt in enumerate(jts):
                                nc.tensor.matmul(
                                    ps,
                                    lhsT=ygs[:, i, ts(tt, 128)],
                                    rhs=wp[:, jt, ts(cc, 512)],
                                    start=(i == 0),
                                    stop=(i == 3),
                                )
                            if w == 0:
                                nc.vector.tensor_add(
                                    stash[:, st, :], ps, bp_bc[:, ts(cc, 512)]
                                )
                            elif w < 3:
                                nc.vector.tensor_add(
                                    stash[:, st, :], ps, stash[:, st, :]
                                )
                            else:
                                o_sb = work.tile(
                                    [128, 512], f32, tag="o_sb", name="o_sb",
                                    bufs=3,
                                )
                                nc.vector.tensor_add(o_sb, ps, stash[:, st, :])
                                nc.sync.dma_start(
                                    out=out.ap()[ts(tt, 128), ts(cc, 512)],
                                    in_=o_sb,
                                )

    # Pin every activation to the one table set holding Exp+Ln+Identity
    # (index 6, natural_log_exp_and_others). Без этого the set-picker
    # alternates exp_and_others <-> natural_log per instruction and inserts
    # ~33 ACT_TABLE_LOADs (~50us of Scalar-engine time).
    import concourse.bacc as bacc_mod

    orig_tables = bacc_mod.get_activation_tables

    def _pinned_tables(arch):
        tabs = orig_tables(arch)
        return {
            name: (funcs if name == "natural_log_exp_and_others" else set())
            for name, funcs in tabs.items()
        }

    bacc_mod.get_activation_tables = _pinned_tables
    try:
        nc.finalize()
    finally:
        bacc_mod.get_activation_tables = orig_tables
    return nc


def _host_inputs(x, w_attn, b_attn, w_proj, b_proj):
    """Build the 8 per-core input maps."""
    x = np.asarray(x, np.float32)
    w_attn = np.asarray(w_attn, np.float32)
    b_attn = np.asarray(b_attn, np.float32)
    w_proj = np.asarray(w_proj, np.float32)
    b_proj = np.asarray(b_proj, np.float32)

    # rope tables, transposed [d, t], full height with rotate-half signs folded:
    # out = x * cos2 + swap_halves(x) * sin2,  cos2=[cos;cos], sin2=[-sin;sin]
    inv_freq = 1.0 / (ROPE_BASE ** (np.arange(0, D, 2, dtype=np.float32) / D))
    freqs = np.arange(T, dtype=np.float32)[:, None] * inv_freq[None, :]  # [T, 64]
    c_ = np.ascontiguousarray(np.cos(freqs).T)  # [64, T]
    s_ = np.ascontiguousarray(np.sin(freqs).T)
    cosT = np.concatenate([c_, c_], axis=0).astype(BF16)  # [128, T]
    sinT = np.concatenate([-s_, s_], axis=0).astype(BF16)

    # causal mask blocks, transposed [k, q]: block kt_rel r, q chunk of 512
    k_idx = np.arange(128)
    q_idx = np.arange(512)
    maskT = np.zeros((128, 4, 512), np.float32)
    for r in range(4):
        maskT[:, r, :] = ((r * 128 + k_idx)[:, None] <= q_idx[None, :]).astype(
            np.float32
        )
    maskT = maskT.astype(BF16)

    permM = np.zeros((128, 128), np.float32)
    permM[(np.arange(128) + 64) % 128, np.arange(128)] = 1.0
    permM = permM.astype(BF16)

    in_maps = []
    for c in range(N_CORES):
        b, g = divmod(c, 2)
        cs = slice(g * 1024, (g + 1) * 1024)
        wq = w_attn[:, 0:C][:, cs]
        wk = w_attn[:, C : 2 * C][:, cs]
        wv = w_attn[:, 2 * C : 3 * C][:, cs]
        bq = b_attn[0:C][cs]
        bk = b_attn[C : 2 * C][cs]
        bvv = b_attn[2 * C : 3 * C][cs]
        in_maps.append(
            {
                "xT": np.ascontiguousarray(x[b].T).astype(BF16),
                "wqkv": np.concatenate([wq, wk, wv], axis=1).astype(BF16),
                "bqk": np.ascontiguousarray(
                    np.concatenate([bq, bk]).reshape(16, 128).T
                ).astype(np.float32),
                "bv": bvv.reshape(1, 1024).astype(np.float32),
                "cosT": cosT,
                "sinT": sinT,
                "maskT": maskT,
                "perm": permM,
                "wproj": w_proj[:, cs].astype(BF16),
                "bproj": b_proj[cs].reshape(1, 1024).astype(np.float32),
            }
        )
    return in_maps


def kernel(x, w_attn, b_attn, w_proj, b_proj, _trace=False):
    from concourse.bass_utils import run_bass_kernel_spmd

    if "nc" not in _PROGRAM_CACHE:
        _PROGRAM_CACHE["nc"] = _build_program()
    nc = _PROGRAM_CACHE["nc"]

    in_maps = _host_inputs(x, w_attn, b_attn, w_proj, b_proj)
    res = run_bass_kernel_spmd(
        nc, in_maps, core_ids=list(range(N_CORES)), trace=_trace
    )
    _PROGRAM_CACHE["last_results"] = res

    out = np.zeros((B, T, C), np.float32)
    for c in range(N_CORES):
        b, g = divmod(c, 2)
        out[b, :, g * 1024 : (g + 1) * 1024] = res.results[c]["out"]
    return out
